# revision 27
# baseline (speedup 1.0000x reference)
# Trainium2 Bass kernel for AvaAttention (GQA attention + RoPE + additive mask)
# B=2, T=2048, HID=2048, NH=16, KVH=4, HD=128, fp32 in/out — 8 NeuronCores.
#
# Sharding: sequence-parallel. Core i (batch b=i//4, position p=i%4) owns
# q-blocks j = 4s+3-p of batch b, for slot s in 0..3. Projections are
# row-parallel (weights replicated, host-cast to bf16), K/V exchanged with
# an AllGather over each batch's 4 cores (bf16, K issued before the V
# projection so the collective overlaps V+Q compute), attention + output
# projection stay local to the core's rows. All matmuls run in bf16 with
# fp32 PSUM accumulation; independent accumulation chains are interleaved
# across PSUM banks so PE drains overlap the next stream.
#
# Attention keeps scores in [q, src] orientation so softmax sums ride the
# scalar activation's accumulator: exp(S)/sum(exp(S)) without
# max-subtraction (safe at this score scale; masked positions hit
# exp(S-1e9)=0). The probability transpose and 1/Z normalization are fused
# into one diag-matmul per block (P^T @ diag(1/Z)), whose PSUM output is
# cast to bf16 and streamed into the PV accumulation. The additive mask is
# applied as *data* via identity-matmul accumulation at mask-plan
# positions that are uniform across cores, so one compiled SPMD program
# serves all 8 cores. Gathered K/V blocks are stored in producer-permuted
# order c = 4*s_src + pos (block j = 4*s_src + 3 - pos), keeping the
# causal prefix contiguous and the gather DMAs full-row. Wo is preloaded
# into SBUF during attention and its matmuls are slot-interleaved across
# PSUM banks.

import sys

for _p in ("/opt/trn_rl_repo", "/opt/pypackages"):
    if _p not in sys.path:
        sys.path.insert(0, _p)

import numpy as np
import ml_dtypes

B, T, HID = 2, 2048, 2048
NH, KVH, HD = 16, 4, 128
P = 128
NC = 8
NBLK = T // P          # 16 q-blocks per batch
NSLOT = 4              # blocks per core
GPQ = NH // KVH        # 4 q-heads per kv group
HB = HID // P          # 16 contraction subtiles
NEG_THRESH = -1.0e8


def _c_of_j(j):
    # producer-permuted column-block index (involution)
    return 4 * (j // 4) + 3 - (j % 4)


def _mask_plan(attention_mask):
    """Classify the additive mask per (j, kb) 128x128 tile.

    Returns (E, P_list): E[s] is the uniform extent (in permuted blocks c)
    for slot s; P_list is the ordered list of (s, c) positions where a
    mask-add is applied (positions shared by every core; tile *data* is
    per-core).
    """
    m = np.asarray(attention_mask).reshape(T, T)
    nonzero = np.zeros((NBLK, NBLK), dtype=bool)
    live = np.zeros((NBLK, NBLK), dtype=bool)   # not fully masked
    for j in range(NBLK):
        for kb in range(NBLK):
            tile = m[j * P:(j + 1) * P, kb * P:(kb + 1) * P]
            nonzero[j, kb] = bool(np.any(tile != 0.0))
            live[j, kb] = bool(np.any(tile > NEG_THRESH))
    E = []
    for s in range(NSLOT):
        cmax = 1
        for jj in range(4):
            j = 4 * s + jj
            idx = np.nonzero(live[j])[0]
            if len(idx):
                cmax = max(cmax, max(_c_of_j(int(kb)) for kb in idx) + 1)
        E.append(cmax)
    P_list = []
    for s in range(NSLOT):
        for c in range(E[s]):
            kb = _c_of_j(c)
            if any(nonzero[4 * s + jj, kb] for jj in range(4)):
                P_list.append((s, c))
    return E, P_list


def _build_program(E, P_list):
    import concourse.mybir as mybir
    import concourse.tile as tile
    from concourse import bacc
    from concourse.masks import make_identity
    from contextlib import ExitStack

    FP32 = mybir.dt.float32
    FP32R = mybir.dt.float32r
    BF16 = mybir.dt.bfloat16
    Exp = mybir.ActivationFunctionType.Exp
    HALF = HD // 2

    nc = bacc.Bacc("TRN2", target_bir_lowering=False, num_devices=NC)

    x_p = nc.declare_dram_parameter("x", [NSLOT * P, HID], BF16, isOutput=False)
    wq_p = nc.declare_dram_parameter("wq", [HID, NH * HD], BF16, isOutput=False)
    wk_p = nc.declare_dram_parameter("wk", [HID, KVH * HD], BF16, isOutput=False)
    wv_p = nc.declare_dram_parameter("wv", [HID, KVH * HD], BF16, isOutput=False)
    wo_p = nc.declare_dram_parameter("wo", [HID, HID], BF16, isOutput=False)
    cosq_p = nc.declare_dram_parameter("cosq", [NSLOT * P, HD], FP32, isOutput=False)
    sinq_p = nc.declare_dram_parameter("sinq3", [NSLOT * P, HD], FP32, isOutput=False)
    cosk_p = nc.declare_dram_parameter("cosk", [NSLOT * P, HD], FP32, isOutput=False)
    sink_p = nc.declare_dram_parameter("sink3", [NSLOT * P, HD], FP32, isOutput=False)
    nmask = max(1, len(P_list))
    masks_p = nc.declare_dram_parameter("masks", [nmask, P, P], BF16, isOutput=False)
    out_p = nc.declare_dram_parameter("out", [NSLOT * P, HID], FP32, isOutput=True)

    KVW = KVH * HD  # 512
    ag_k_in = nc.dram_tensor("ag_k_in", [KVW, NSLOT * P], BF16)
    ag_k_out = nc.dram_tensor("ag_k_out", [4, KVW, NSLOT * P], BF16, addr_space="Local")
    ag_v_in = nc.dram_tensor("ag_v_in", [NSLOT * P, KVW], BF16)
    ag_v_out = nc.dram_tensor("ag_v_out", [4, NSLOT * P, KVW], BF16, addr_space="Local")
    groups = [[0, 1, 2, 3], [4, 5, 6, 7]]

    mask_idx = {sk: idx for idx, sk in enumerate(P_list)}

    def rope(engine, dst, src_ps, cos_t, sin_t, s, nh):
        """dst[t, h, d] = src*cos + rotate_half(src)*sin, natural layout."""
        src3 = src_ps[:].rearrange("p (h d) -> p h d", d=HD)
        cst = rope.pool.tile([P, nh, HD], FP32, name="rope_c", tag="rope_c")
        engine.tensor_tensor(dst[:], src3,
                             cos_t[:, s, None, :].to_broadcast((P, nh, HD)),
                             mybir.AluOpType.mult)
        engine.tensor_tensor(cst[:], src3,
                             sin_t[:, s, None, :].to_broadcast((P, nh, HD)),
                             mybir.AluOpType.mult)
        engine.tensor_tensor(dst[:, :, HALF:], dst[:, :, HALF:],
                             cst[:, :, :HALF], mybir.AluOpType.add)
        engine.tensor_tensor(dst[:, :, :HALF], dst[:, :, :HALF],
                             cst[:, :, HALF:], mybir.AluOpType.add)

    with tile.TileContext(nc) as tc, ExitStack() as top:
        const = top.enter_context(tc.tile_pool(name="const", bufs=1))
        ident_bf = const.tile([P, P], BF16)
        make_identity(nc, ident_bf[:])
        ident_f32 = const.tile([P, P], FP32)
        make_identity(nc, ident_f32[:])
        masks_t = const.tile([P, nmask, P], BF16)

        qT_pool = top.enter_context(tc.tile_pool(name="qT_pool", bufs=1))
        qT = qT_pool.tile([P, NH, NSLOT * P], BF16)           # [d, h, t]

        # ================= projection phases =================
        with tc.tile_pool(name="xT_pool", bufs=1) as xT_pool, \
             tc.tile_pool(name="ropec", bufs=1) as ropec, \
             tc.tile_pool(name="qw", bufs=2) as qw_pool:
            cosq_t = ropec.tile([P, NSLOT, HD], FP32)
            sinq_t = ropec.tile([P, NSLOT, HD], FP32)
            cosk_t = ropec.tile([P, NSLOT, HD], FP32)
            sink_t = ropec.tile([P, NSLOT, HD], FP32)

            xT = xT_pool.tile([P, HB, NSLOT * P], BF16)       # [h%128, hb, t]

            # ---- phase 0: load x, transpose to xT ----
            with tc.tile_pool(name="xph", bufs=2) as xpool, \
                 tc.tile_pool(name="xps", bufs=2, space="PSUM") as xps:
                x_nat = []
                for s in range(NSLOT):
                    xs = xpool.tile([P, HID], BF16, name=f"x_nat{s}", tag=f"x_nat{s % 2}")
                    nc.sync.dma_start(xs[:], x_p[s * P:(s + 1) * P, :])
                    x_nat.append(xs)
                for hb in range(HB):
                    pxt = xps.tile([P, NSLOT * P], BF16, name="pxt", tag="pxt")
                    for s in range(NSLOT):
                        nc.tensor.transpose(pxt[:, s * P:(s + 1) * P],
                                            x_nat[s][:, hb * P:(hb + 1) * P],
                                            ident_bf[:])
                    nc.vector.tensor_copy(xT[:, hb, :], pxt[:])

            # ---- phase 1a: K proj, V proj, RoPE, AllGathers ----
            QC = 4  # heads per Wq chunk
            with tc.tile_pool(name="kvw", bufs=1) as kvw_pool, \
                 tc.tile_pool(name="kvstage", bufs=2) as kvstage, \
                 tc.tile_pool(name="kvps", bufs=1, space="PSUM") as kvps:
                rope.pool = kvstage
                wk_sb = kvw_pool.tile([P, HB, KVW], BF16, name="wk_sb")
                wv_sb = kvw_pool.tile([P, HB, KVW], BF16, name="wv_sb")
                nc.sync.dma_start(wk_sb[:], wk_p[:]
                                  .rearrange("(hb p) n -> p hb n", p=P))
                nc.sync.dma_start(wv_sb[:], wv_p[:]
                                  .rearrange("(hb p) n -> p hb n", p=P))
                # prefetch the first two Wq chunks behind K/V weights
                wq_tiles = {}
                for hc in range(2):
                    wq_sb = qw_pool.tile([P, HB, QC * HD], BF16,
                                         name="wq_sb", tag="wq")
                    nc.sync.dma_start(
                        wq_sb[:],
                        wq_p[:, hc * QC * HD:(hc + 1) * QC * HD]
                        .rearrange("(hb p) n -> p hb n", p=P))
                    wq_tiles[hc] = wq_sb
                for ap, prm in ((cosk_t, cosk_p), (sink_t, sink_p),
                                (cosq_t, cosq_p), (sinq_t, sinq_p)):
                    nc.sync.dma_start(ap[:], prm[:].rearrange("(s p) d -> p s d", p=P))
                nc.sync.dma_start(masks_t[:], masks_p[:].rearrange("n p d -> p n d"))

                contrib_k = kvw_pool.tile([P, KVH, NSLOT * P], BF16, name="contrib_k")
                pk = [kvps.tile([P, KVW], FP32, name=f"pk{s}", tag=f"pkv{s}")
                      for s in range(NSLOT)]
                for hb in range(HB):
                    for s in range(NSLOT):
                        nc.tensor.matmul(pk[s][:], xT[:, hb, s * P:(s + 1) * P],
                                         wk_sb[:, hb, :],
                                         start=(hb == 0), stop=(hb == HB - 1))
                k_rope = []
                for s in range(NSLOT):
                    kr = kvw_pool.tile([P, KVH, HD], FP32, name=f"k_rope{s}")
                    rope(nc.vector, kr, pk[s], cosk_t, sink_t, s, KVH)
                    k_rope.append(kr)
                # V projection streams while the K rope tail runs on vector
                pv = [kvps.tile([P, KVW], FP32, name=f"pv{s}", tag=f"pkv{s}")
                      for s in range(NSLOT)]
                for hb in range(HB):
                    for s in range(NSLOT):
                        nc.tensor.matmul(pv[s][:], xT[:, hb, s * P:(s + 1) * P],
                                         wv_sb[:, hb, :],
                                         start=(hb == 0), stop=(hb == HB - 1))
                for g in range(KVH):
                    pkt = kvps.tile([P, NSLOT * P], FP32, name="pkt",
                                    tag=f"pkt{g % 2}")
                    for s in range(NSLOT):
                        nc.tensor.transpose(pkt[:, s * P:(s + 1) * P],
                                            k_rope[s][:, g, :], ident_f32[:])
                    nc.vector.tensor_copy(contrib_k[:, g, :], pkt[:])
                nc.sync.dma_start(
                    ag_k_in[:].rearrange("(g d) t -> d g t", d=P), contrib_k[:])

                nc.gpsimd.collective_compute(
                    "AllGather", mybir.AluOpType.bypass, replica_groups=groups,
                    ins=[ag_k_in[:]], outs=[ag_k_out[:]])

                for s in range(NSLOT):
                    vst = kvstage.tile([P, KVW], BF16, name=f"v_st{s}", tag="v_st")
                    nc.vector.tensor_copy(vst[:], pv[s][:])
                    nc.sync.dma_start(ag_v_in[s * P:(s + 1) * P, :], vst[:])

                nc.gpsimd.collective_compute(
                    "AllGather", mybir.AluOpType.bypass, replica_groups=groups,
                    ins=[ag_v_in[:]], outs=[ag_v_out[:]])

            # ---- phase 1b: Q projection + RoPE + transpose to qT ----
            with tc.tile_pool(name="qstage", bufs=3) as qstage, \
                 tc.tile_pool(name="qps", bufs=1, space="PSUM") as qps, \
                 tc.tile_pool(name="qtps", bufs=2, space="PSUM") as qtps:
                rope.pool = qstage
                for hc in range(NH // QC):
                    if hc in wq_tiles:
                        wq_sb = wq_tiles[hc]
                    else:
                        wq_sb = qw_pool.tile([P, HB, QC * HD], BF16,
                                             name="wq_sb", tag="wq")
                        nc.sync.dma_start(
                            wq_sb[:],
                            wq_p[:, hc * QC * HD:(hc + 1) * QC * HD]
                            .rearrange("(hb p) n -> p hb n", p=P))
                    pq = [qps.tile([P, QC * HD], FP32, name=f"pq{s}",
                                   tag=f"pq{s}") for s in range(NSLOT)]
                    for hb in range(HB):
                        for s in range(NSLOT):
                            nc.tensor.matmul(pq[s][:], xT[:, hb, s * P:(s + 1) * P],
                                             wq_sb[:, hb, :],
                                             start=(hb == 0), stop=(hb == HB - 1))
                    q_rope = []
                    for s in range(NSLOT):
                        qr = qstage.tile([P, QC, HD], FP32, name=f"q_rope{s}",
                                         tag=f"q_rope{s % 2}")
                        rope(nc.vector, qr, pq[s], cosq_t, sinq_t, s, QC)
                        q_rope.append(qr)
                    for h in range(QC):
                        pqt = qtps.tile([P, NSLOT * P], FP32, name="pqt", tag="pqt")
                        for s in range(NSLOT):
                            nc.tensor.transpose(pqt[:, s * P:(s + 1) * P],
                                                q_rope[s][:, h, :], ident_f32[:])
                        nc.vector.tensor_copy(qT[:, hc * QC + h, :], pqt[:])

        # ================= gather + attention + output =================
        with tc.tile_pool(name="kv_pool", bufs=1) as kv_pool, \
             tc.tile_pool(name="wo_pool", bufs=1) as wo_pool:
            # permuted block order: column block c = 4*s_src + pos holds
            # t-block j = 4*s_src + 3 - pos
            kT = kv_pool.tile([P, KVH, NSLOT, 4, P], BF16)    # [d, g, s_src, pos, t]
            v_all = kv_pool.tile([P, NBLK, KVW], BF16)        # [t%128, c, (g d)]

            for g in range(KVH):
                for pos in range(4):
                    nc.sync.dma_start(
                        kT[:, g, :, pos, :],
                        ag_k_out[pos, g * P:(g + 1) * P, :]
                        .rearrange("d (s t) -> d s t", t=P))
            for s_src in range(NSLOT):
                for pos in range(4):
                    nc.sync.dma_start(v_all[:, 4 * s_src + pos, :],
                                      ag_v_out[pos, s_src * P:(s_src + 1) * P, :])

            kTf = kT[:].rearrange("d g s q t -> d g (s q t)")

            # Wo halves preloaded here so the DMA overlaps attention
            OC = 1024
            wo_a = wo_pool.tile([P, HB, OC], BF16, name="wo_a")
            wo_b = wo_pool.tile([P, HB, OC], BF16, name="wo_b")
            for oci, wt in ((0, wo_a), (1, wo_b)):
                nc.sync.dma_start(
                    wt[:],
                    wo_p[:, oci * OC:(oci + 1) * OC]
                    .rearrange("(hb p) n -> p hb n", p=P))

            with tc.tile_pool(name="ctxT_pool", bufs=1) as ctxT_pool:
                ctxT = ctxT_pool.tile([P, NSLOT, KVH, GPQ, P], BF16)

                # ---- phase 3: attention ----
                CH = 512 // P
                with tc.tile_pool(name="ppool", bufs=2) as ppool, \
                     tc.tile_pool(name="astage", bufs=4) as astage, \
                     tc.tile_pool(name="ptsp", bufs=6) as ptsp, \
                     tc.tile_pool(name="dstage", bufs=3) as dstage, \
                     tc.tile_pool(name="sps", bufs=5, space="PSUM") as sps, \
                     tc.tile_pool(name="tps", bufs=2, space="PSUM") as tps, \
                     tc.tile_pool(name="cps", bufs=1, space="PSUM") as cps:
                    for s in range(NSLOT):
                        Es = E[s]
                        nch = (Es + CH - 1) // CH
                        for g in range(KVH):
                            sums = astage.tile([P, GPQ * nch], FP32,
                                               name="sums", tag="sums")
                            p_tiles = [ppool.tile([P, Es * P], BF16,
                                                  name=f"p_{h}", tag=f"p_{h}")
                                       for h in range(GPQ)]
                            for c in range(nch):
                                k0, k1 = c * CH, min(Es, (c + 1) * CH)
                                ncols = (k1 - k0) * P
                                adds = [kb for kb in range(k0, k1)
                                        if (s, kb) in mask_idx]
                                for h in range(GPQ):
                                    pss = sps.tile([P, 512], FP32,
                                                   name="pss", tag="pss")
                                    nc.tensor.matmul(
                                        pss[:, :ncols],
                                        qT[:, g * GPQ + h, s * P:(s + 1) * P],
                                        kTf[:, g, k0 * P:k1 * P],
                                        start=True, stop=(not adds))
                                    for na, kb in enumerate(adds):
                                        mi = mask_idx[(s, kb)]
                                        nc.tensor.matmul(
                                            pss[:, (kb - k0) * P:(kb - k0 + 1) * P],
                                            ident_bf[:], masks_t[:, mi, :],
                                            start=False, stop=(na == len(adds) - 1))
                                    nc.scalar.activation(
                                        p_tiles[h][:, k0 * P:k1 * P],
                                        pss[:, :ncols], Exp,
                                        accum_out=sums[:, h * nch + c:
                                                       h * nch + c + 1])
                            rs = astage.tile([P, GPQ], FP32, name="rs", tag="rs")
                            nc.vector.tensor_reduce(
                                rs[:],
                                sums[:].rearrange("p (h c) -> p h c", c=nch),
                                axis=mybir.AxisListType.X,
                                op=mybir.AluOpType.add)
                            rr = astage.tile([P, GPQ], FP32, name="rr", tag="rr")
                            nc.vector.reciprocal(rr[:], rs[:])
                            diags = []
                            for h in range(GPQ):
                                dg = dstage.tile([P, P], BF16,
                                                 name=f"diag{h}", tag=f"diag{h}")
                                nc.vector.tensor_scalar_mul(dg[:], ident_bf[:],
                                                            rr[:, h:h + 1])
                                diags.append(dg)
                            pctx = cps.tile([P, GPQ * P], FP32,
                                            name="pctx", tag="pctx")
                            for kb in range(Es):
                                ppt = tps.tile([P, GPQ * P], FP32,
                                               name="ppt", tag="ppt")
                                for h in range(GPQ):
                                    nc.tensor.matmul(
                                        ppt[:, h * P:(h + 1) * P],
                                        p_tiles[h][:, kb * P:(kb + 1) * P],
                                        diags[h], start=True, stop=True)
                                pts = ptsp.tile([P, GPQ * P], BF16,
                                                name="pts", tag="pts")
                                nc.vector.tensor_copy(pts[:], ppt[:])
                                nc.tensor.matmul(pctx[:],
                                                 v_all[:, kb, g * HD:(g + 1) * HD],
                                                 pts[:],
                                                 start=(kb == 0),
                                                 stop=(kb == Es - 1))
                            nc.vector.tensor_copy(
                                ctxT[:, s, g, :, :],
                                pctx[:].rearrange("p (h d) -> p h d", d=P))

                # ---- phase 4: output projection ----
                with tc.tile_pool(name="ostage", bufs=3) as ostage, \
                     tc.tile_pool(name="ops", bufs=1, space="PSUM") as ops:
                    for oci, wt in ((0, wo_a), (1, wo_b)):
                        for sub in range(OC // 512):
                            po = [ops.tile([P, 512], FP32, name=f"po{s}",
                                           tag=f"po{s}") for s in range(NSLOT)]
                            for g in range(KVH):
                                for h in range(GPQ):
                                    hh = g * GPQ + h
                                    for s in range(NSLOT):
                                        nc.tensor.matmul(
                                            po[s][:], ctxT[:, s, g, h, :],
                                            wt[:, hh, sub * 512:(sub + 1) * 512],
                                            start=(hh == 0),
                                            stop=(hh == HB - 1))
                            for s in range(NSLOT):
                                ot = ostage.tile([P, 512], FP32, name="ot",
                                                 tag=f"ot{s % 2}")
                                nc.vector.tensor_copy(ot[:], po[s][:])
                                nc.sync.dma_start(
                                    out_p[s * P:(s + 1) * P,
                                          oci * OC + sub * 512:
                                          oci * OC + (sub + 1) * 512],
                                    ot[:])

    nc.compile()
    return nc


def _prep_inputs(hidden_states, attention_mask, cos, sin, Wq, Wk, Wv, Wo, P_list):
    bf16 = ml_dtypes.bfloat16
    hs = np.asarray(hidden_states, dtype=np.float32).astype(bf16)
    mask = np.asarray(attention_mask, dtype=np.float32).reshape(T, T)
    cos2 = np.asarray(cos, dtype=np.float32).reshape(T, HD)
    sin2 = np.asarray(sin, dtype=np.float32).reshape(T, HD)
    scale = np.float32(1.0 / np.sqrt(HD))

    def t3(s_):
        # rotate_half add trick: t3 = concat(sin[:, 64:], -sin[:, :64])
        return np.concatenate([s_[:, HD // 2:], -s_[:, :HD // 2]], axis=1)

    wq = np.ascontiguousarray(np.asarray(Wq, dtype=np.float32)).astype(bf16)
    wk = np.ascontiguousarray(np.asarray(Wk, dtype=np.float32)).astype(bf16)
    wv = np.ascontiguousarray(np.asarray(Wv, dtype=np.float32)).astype(bf16)
    wo = np.ascontiguousarray(np.asarray(Wo, dtype=np.float32)).astype(bf16)

    in_maps = []
    for i in range(NC):
        b, pos = i // 4, i % 4
        js = [4 * s + 3 - pos for s in range(NSLOT)]
        take = lambda a: np.ascontiguousarray(
            np.concatenate([a[j * P:(j + 1) * P] for j in js], axis=0))
        m_tiles = [mask[js[s] * P:(js[s] + 1) * P,
                        _c_of_j(c) * P:(_c_of_j(c) + 1) * P]
                   for (s, c) in P_list]
        if not m_tiles:
            m_tiles.append(np.zeros((P, P), np.float32))
        in_maps.append({
            "x": take(hs[b]),
            "wq": wq, "wk": wk, "wv": wv, "wo": wo,
            "cosq": take(cos2 * scale),
            "sinq3": take(t3(sin2 * scale)),
            "cosk": take(cos2),
            "sink3": take(t3(sin2)),
            "masks": np.stack([np.asarray(m, np.float32) for m in m_tiles]
                              ).astype(bf16),
        })
    return in_maps


_cache = {}


def kernel(hidden_states, attention_mask, cos, sin, Wq, Wk, Wv, Wo,
           _trace=False, _trace_kwargs=None):
    from concourse.bass_utils import run_bass_kernel_spmd

    E, P_list = _mask_plan(attention_mask)
    key = (tuple(E), tuple(P_list))
    if key not in _cache:
        _cache[key] = _build_program(E, P_list)
    nc = _cache[key]

    in_maps = _prep_inputs(hidden_states, attention_mask, cos, sin,
                           Wq, Wk, Wv, Wo, P_list)
    kwargs = dict(_trace_kwargs or {})
    if _trace:
        kwargs["trace"] = True
    res = run_bass_kernel_spmd(nc, in_maps, list(range(NC)), **kwargs)

    out = np.empty((B, T, HID), dtype=np.float32)
    for i in range(NC):
        b, pos = i // 4, i % 4
        o = res.results[i]["out"]
        for s in range(NSLOT):
            j = 4 * s + 3 - pos
            out[b, j * P:(j + 1) * P, :] = o[s * P:(s + 1) * P, :]
    kernel._last_result = res
    return out


# revision 29
# speedup vs baseline: 1.1311x; 1.1311x over previous
# Trainium2 Bass kernel for AvaAttention (GQA attention + RoPE + additive mask)
# B=2, T=2048, HID=2048, NH=16, KVH=4, HD=128, fp32 in/out — 8 NeuronCores.
#
# Sharding: sequence-parallel. Core i (batch b=i//4, position p=i%4) owns
# q-blocks j = 4s+3-p of batch b, for slot s in 0..3. Projections are
# row-parallel (weights replicated, host-cast to bf16), K/V exchanged with
# an AllGather over each batch's 4 cores (bf16, K issued before the V
# projection so the collective overlaps V+Q compute), attention + output
# projection stay local to the core's rows. All matmuls run in bf16 with
# fp32 PSUM accumulation; independent accumulation chains are interleaved
# across PSUM banks so PE drains overlap the next stream.
#
# Attention keeps scores in [q, src] orientation so softmax sums ride the
# scalar activation's accumulator: exp(S)/sum(exp(S)) without
# max-subtraction (safe at this score scale; masked positions hit
# exp(S-1e9)=0). The probability transpose and 1/Z normalization are fused
# into one diag-matmul per block (P^T @ diag(1/Z)), whose PSUM output is
# cast to bf16 and streamed into the PV accumulation. The additive mask is
# applied as *data* via identity-matmul accumulation at mask-plan
# positions that are uniform across cores, so one compiled SPMD program
# serves all 8 cores. Gathered K/V blocks are stored in producer-permuted
# order c = 4*s_src + pos (block j = 4*s_src + 3 - pos), keeping the
# causal prefix contiguous and the gather DMAs full-row. Wo is preloaded
# into SBUF during attention and its matmuls are slot-interleaved across
# PSUM banks.

import sys

for _p in ("/opt/trn_rl_repo", "/opt/pypackages"):
    if _p not in sys.path:
        sys.path.insert(0, _p)

import numpy as np
import ml_dtypes

B, T, HID = 2, 2048, 2048
NH, KVH, HD = 16, 4, 128
P = 128
NC = 8
NBLK = T // P          # 16 q-blocks per batch
NSLOT = 4              # blocks per core
GPQ = NH // KVH        # 4 q-heads per kv group
HB = HID // P          # 16 contraction subtiles
NEG_THRESH = -1.0e8


def _c_of_j(j):
    # producer-permuted column-block index (involution)
    return 4 * (j // 4) + 3 - (j % 4)


def _mask_plan(attention_mask):
    """Classify the additive mask per (j, kb) 128x128 tile.

    Returns (E, P_list): E[s] is the uniform extent (in permuted blocks c)
    for slot s; P_list is the ordered list of (s, c) positions where a
    mask-add is applied (positions shared by every core; tile *data* is
    per-core).
    """
    m = np.asarray(attention_mask).reshape(T, T)
    nonzero = np.zeros((NBLK, NBLK), dtype=bool)
    live = np.zeros((NBLK, NBLK), dtype=bool)   # not fully masked
    for j in range(NBLK):
        for kb in range(NBLK):
            tile = m[j * P:(j + 1) * P, kb * P:(kb + 1) * P]
            nonzero[j, kb] = bool(np.any(tile != 0.0))
            live[j, kb] = bool(np.any(tile > NEG_THRESH))
    E = []
    for s in range(NSLOT):
        cmax = 1
        for jj in range(4):
            j = 4 * s + jj
            idx = np.nonzero(live[j])[0]
            if len(idx):
                cmax = max(cmax, max(_c_of_j(int(kb)) for kb in idx) + 1)
        E.append(cmax)
    P_list = []
    for s in range(NSLOT):
        for c in range(E[s]):
            kb = _c_of_j(c)
            if any(nonzero[4 * s + jj, kb] for jj in range(4)):
                P_list.append((s, c))
    return E, P_list


def _build_program(E, P_list):
    import concourse.mybir as mybir
    import concourse.tile as tile
    from concourse import bacc
    from concourse.masks import make_identity
    from contextlib import ExitStack

    FP32 = mybir.dt.float32
    FP32R = mybir.dt.float32r
    BF16 = mybir.dt.bfloat16
    Exp = mybir.ActivationFunctionType.Exp
    HALF = HD // 2

    nc = bacc.Bacc("TRN2", target_bir_lowering=False, num_devices=NC)

    x_p = nc.declare_dram_parameter("x", [NSLOT * P, HID], BF16, isOutput=False)
    wq_p = nc.declare_dram_parameter("wq", [HID, NH * HD], BF16, isOutput=False)
    wk_p = nc.declare_dram_parameter("wk", [HID, KVH * HD], BF16, isOutput=False)
    wv_p = nc.declare_dram_parameter("wv", [HID, KVH * HD], BF16, isOutput=False)
    wo_p = nc.declare_dram_parameter("wo", [HID, HID], BF16, isOutput=False)
    cosq_p = nc.declare_dram_parameter("cosq", [NSLOT * P, HD], FP32, isOutput=False)
    sinq_p = nc.declare_dram_parameter("sinq3", [NSLOT * P, HD], FP32, isOutput=False)
    cosk_p = nc.declare_dram_parameter("cosk", [NSLOT * P, HD], FP32, isOutput=False)
    sink_p = nc.declare_dram_parameter("sink3", [NSLOT * P, HD], FP32, isOutput=False)
    nmask = max(1, len(P_list))
    masks_p = nc.declare_dram_parameter("masks", [nmask, P, P], BF16, isOutput=False)
    out_p = nc.declare_dram_parameter("out", [NSLOT * P, HID], FP32, isOutput=True)

    KVW = KVH * HD  # 512
    ag_k_in = nc.dram_tensor("ag_k_in", [KVW, NSLOT * P], BF16)
    ag_k_out = nc.dram_tensor("ag_k_out", [4, KVW, NSLOT * P], BF16, addr_space="Local")
    ag_v_in = nc.dram_tensor("ag_v_in", [NSLOT * P, KVW], BF16)
    ag_v_out = nc.dram_tensor("ag_v_out", [4, NSLOT * P, KVW], BF16, addr_space="Local")
    groups = [[0, 1, 2, 3], [4, 5, 6, 7]]

    mask_idx = {sk: idx for idx, sk in enumerate(P_list)}

    def rope(engine, dst, src_ps, cos_t, sin_t, s, nh):
        """dst[t, h, d] = src*cos + rotate_half(src)*sin, natural layout."""
        src3 = src_ps[:].rearrange("p (h d) -> p h d", d=HD)
        cst = rope.pool.tile([P, nh, HD], FP32, name="rope_c", tag="rope_c")
        engine.tensor_tensor(dst[:], src3,
                             cos_t[:, s, None, :].to_broadcast((P, nh, HD)),
                             mybir.AluOpType.mult)
        engine.tensor_tensor(cst[:], src3,
                             sin_t[:, s, None, :].to_broadcast((P, nh, HD)),
                             mybir.AluOpType.mult)
        engine.tensor_tensor(dst[:, :, HALF:], dst[:, :, HALF:],
                             cst[:, :, :HALF], mybir.AluOpType.add)
        engine.tensor_tensor(dst[:, :, :HALF], dst[:, :, :HALF],
                             cst[:, :, HALF:], mybir.AluOpType.add)

    with tile.TileContext(nc) as tc, ExitStack() as top:
        const = top.enter_context(tc.tile_pool(name="const", bufs=1))
        ident_bf = const.tile([P, P], BF16)
        make_identity(nc, ident_bf[:])
        ident_f32 = const.tile([P, P], FP32)
        make_identity(nc, ident_f32[:])
        masks_t = const.tile([P, nmask, P], BF16)

        qT_pool = top.enter_context(tc.tile_pool(name="qT_pool", bufs=1))
        qT = qT_pool.tile([P, NH, NSLOT * P], BF16)           # [d, h, t]

        # ================= projection phases =================
        with tc.tile_pool(name="xT_pool", bufs=1) as xT_pool, \
             tc.tile_pool(name="ropec", bufs=1) as ropec, \
             tc.tile_pool(name="qw", bufs=2) as qw_pool:
            cosq_t = ropec.tile([P, NSLOT, HD], FP32)
            sinq_t = ropec.tile([P, NSLOT, HD], FP32)
            cosk_t = ropec.tile([P, NSLOT, HD], FP32)
            sink_t = ropec.tile([P, NSLOT, HD], FP32)

            xT = xT_pool.tile([P, HB, NSLOT * P], BF16)       # [h%128, hb, t]

            # ---- phase 0: load x, transpose to xT ----
            with tc.tile_pool(name="xph", bufs=2) as xpool, \
                 tc.tile_pool(name="xps", bufs=2, space="PSUM") as xps:
                x_nat = []
                for s in range(NSLOT):
                    xs = xpool.tile([P, HID], BF16, name=f"x_nat{s}", tag=f"x_nat{s % 2}")
                    nc.sync.dma_start(xs[:], x_p[s * P:(s + 1) * P, :])
                    x_nat.append(xs)
                for hb in range(HB):
                    pxt = xps.tile([P, NSLOT * P], BF16, name="pxt", tag="pxt")
                    for s in range(NSLOT):
                        nc.tensor.transpose(pxt[:, s * P:(s + 1) * P],
                                            x_nat[s][:, hb * P:(hb + 1) * P],
                                            ident_bf[:])
                    nc.vector.tensor_copy(xT[:, hb, :], pxt[:])

            # ---- phase 1a: K proj, V proj, RoPE, AllGathers ----
            QC = 4  # heads per Wq chunk
            with tc.tile_pool(name="kvw", bufs=1) as kvw_pool, \
                 tc.tile_pool(name="kvstage", bufs=2) as kvstage, \
                 tc.tile_pool(name="kvps", bufs=1, space="PSUM") as kvps:
                rope.pool = kvstage
                wk_sb = kvw_pool.tile([P, HB, KVW], BF16, name="wk_sb")
                wv_sb = kvw_pool.tile([P, HB, KVW], BF16, name="wv_sb")
                nc.sync.dma_start(wk_sb[:], wk_p[:]
                                  .rearrange("(hb p) n -> p hb n", p=P))
                nc.sync.dma_start(wv_sb[:], wv_p[:]
                                  .rearrange("(hb p) n -> p hb n", p=P))
                # prefetch the first two Wq chunks behind K/V weights
                wq_tiles = {}
                for hc in range(2):
                    wq_sb = qw_pool.tile([P, HB, QC * HD], BF16,
                                         name="wq_sb", tag="wq")
                    nc.sync.dma_start(
                        wq_sb[:],
                        wq_p[:, hc * QC * HD:(hc + 1) * QC * HD]
                        .rearrange("(hb p) n -> p hb n", p=P))
                    wq_tiles[hc] = wq_sb
                for ap, prm in ((cosk_t, cosk_p), (sink_t, sink_p),
                                (cosq_t, cosq_p), (sinq_t, sinq_p)):
                    nc.sync.dma_start(ap[:], prm[:].rearrange("(s p) d -> p s d", p=P))
                nc.sync.dma_start(masks_t[:], masks_p[:].rearrange("n p d -> p n d"))

                contrib_k = kvw_pool.tile([P, KVH, NSLOT * P], BF16, name="contrib_k")
                pk = [kvps.tile([P, KVW], FP32, name=f"pk{s}", tag=f"pkv{s}")
                      for s in range(NSLOT)]
                for hb in range(HB):
                    for s in range(NSLOT):
                        nc.tensor.matmul(pk[s][:], xT[:, hb, s * P:(s + 1) * P],
                                         wk_sb[:, hb, :],
                                         start=(hb == 0), stop=(hb == HB - 1))
                k_rope = []
                for s in range(NSLOT):
                    kr = kvw_pool.tile([P, KVH, HD], FP32, name=f"k_rope{s}")
                    rope(nc.vector, kr, pk[s], cosk_t, sink_t, s, KVH)
                    k_rope.append(kr)
                # V projection streams while the K rope tail runs on vector
                pv = [kvps.tile([P, KVW], FP32, name=f"pv{s}", tag=f"pkv{s}")
                      for s in range(NSLOT)]
                for hb in range(HB):
                    for s in range(NSLOT):
                        nc.tensor.matmul(pv[s][:], xT[:, hb, s * P:(s + 1) * P],
                                         wv_sb[:, hb, :],
                                         start=(hb == 0), stop=(hb == HB - 1))
                for g in range(KVH):
                    pkt = kvps.tile([P, NSLOT * P], FP32, name="pkt",
                                    tag=f"pkt{g % 2}")
                    for s in range(NSLOT):
                        nc.tensor.transpose(pkt[:, s * P:(s + 1) * P],
                                            k_rope[s][:, g, :], ident_f32[:])
                    nc.vector.tensor_copy(contrib_k[:, g, :], pkt[:])
                nc.sync.dma_start(
                    ag_k_in[:].rearrange("(g d) t -> d g t", d=P), contrib_k[:])

                nc.gpsimd.collective_compute(
                    "AllGather", mybir.AluOpType.bypass, replica_groups=groups,
                    ins=[ag_k_in[:]], outs=[ag_k_out[:]])

                for s in range(NSLOT):
                    vst = kvstage.tile([P, KVW], BF16, name=f"v_st{s}", tag="v_st")
                    nc.vector.tensor_copy(vst[:], pv[s][:])
                    nc.sync.dma_start(ag_v_in[s * P:(s + 1) * P, :], vst[:])

                nc.gpsimd.collective_compute(
                    "AllGather", mybir.AluOpType.bypass, replica_groups=groups,
                    ins=[ag_v_in[:]], outs=[ag_v_out[:]])

            # ---- phase 1b: Q projection + RoPE + transpose to qT ----
            with tc.tile_pool(name="qstage", bufs=3) as qstage, \
                 tc.tile_pool(name="qps", bufs=1, space="PSUM") as qps, \
                 tc.tile_pool(name="qtps", bufs=2, space="PSUM") as qtps:
                rope.pool = qstage
                for hc in range(NH // QC):
                    if hc in wq_tiles:
                        wq_sb = wq_tiles[hc]
                    else:
                        wq_sb = qw_pool.tile([P, HB, QC * HD], BF16,
                                             name="wq_sb", tag="wq")
                        nc.sync.dma_start(
                            wq_sb[:],
                            wq_p[:, hc * QC * HD:(hc + 1) * QC * HD]
                            .rearrange("(hb p) n -> p hb n", p=P))
                    pq = [qps.tile([P, QC * HD], FP32, name=f"pq{s}",
                                   tag=f"pq{s}") for s in range(NSLOT)]
                    for hb in range(HB):
                        for s in range(NSLOT):
                            nc.tensor.matmul(pq[s][:], xT[:, hb, s * P:(s + 1) * P],
                                             wq_sb[:, hb, :],
                                             start=(hb == 0), stop=(hb == HB - 1))
                    q_rope = []
                    for s in range(NSLOT):
                        qr = qstage.tile([P, QC, HD], FP32, name=f"q_rope{s}",
                                         tag=f"q_rope{s % 2}")
                        rope(nc.vector, qr, pq[s], cosq_t, sinq_t, s, QC)
                        q_rope.append(qr)
                    for h in range(QC):
                        pqt = qtps.tile([P, NSLOT * P], FP32, name="pqt", tag="pqt")
                        for s in range(NSLOT):
                            nc.tensor.transpose(pqt[:, s * P:(s + 1) * P],
                                                q_rope[s][:, h, :], ident_f32[:])
                        nc.vector.tensor_copy(qT[:, hc * QC + h, :], pqt[:])

        # ================= gather + attention + output =================
        with tc.tile_pool(name="kv_pool", bufs=1) as kv_pool, \
             tc.tile_pool(name="wo_pool", bufs=1) as wo_pool:
            # permuted block order: column block c = 4*s_src + pos holds
            # t-block j = 4*s_src + 3 - pos
            kT = kv_pool.tile([P, KVH, NSLOT, 4, P], BF16)    # [d, g, s_src, pos, t]
            v_all = kv_pool.tile([P, NBLK, KVW], BF16)        # [t%128, c, (g d)]

            for g in range(KVH):
                for pos in range(4):
                    nc.sync.dma_start(
                        kT[:, g, :, pos, :],
                        ag_k_out[pos, g * P:(g + 1) * P, :]
                        .rearrange("d (s t) -> d s t", t=P))
            for s_src in range(NSLOT):
                for pos in range(4):
                    nc.sync.dma_start(v_all[:, 4 * s_src + pos, :],
                                      ag_v_out[pos, s_src * P:(s_src + 1) * P, :])

            kTf = kT[:].rearrange("d g s q t -> d g (s q t)")

            # Wo halves preloaded here so the DMA overlaps attention
            OC = 1024
            wo_a = wo_pool.tile([P, HB, OC], BF16, name="wo_a")
            wo_b = wo_pool.tile([P, HB, OC], BF16, name="wo_b")
            for oci, wt in ((0, wo_a), (1, wo_b)):
                nc.sync.dma_start(
                    wt[:],
                    wo_p[:, oci * OC:(oci + 1) * OC]
                    .rearrange("(hb p) n -> p hb n", p=P))

            with tc.tile_pool(name="ctxT_pool", bufs=1) as ctxT_pool:
                ctxT = ctxT_pool.tile([P, NSLOT, KVH, GPQ, P], BF16)

                # ---- phase 3: attention ----
                CH = 512 // P
                with tc.tile_pool(name="ppool", bufs=2) as ppool, \
                     tc.tile_pool(name="astage", bufs=4) as astage, \
                     tc.tile_pool(name="dstage", bufs=2) as dstage, \
                     tc.tile_pool(name="sps", bufs=4, space="PSUM") as sps, \
                     tc.tile_pool(name="tps", bufs=2, space="PSUM") as tps, \
                     tc.tile_pool(name="cps", bufs=2, space="PSUM") as cps:
                    for s in range(NSLOT):
                        Es = E[s]
                        nch = (Es + CH - 1) // CH
                        for g in range(KVH):
                            sums = astage.tile([P, GPQ * nch], FP32,
                                               name="sums", tag="sums")
                            p_tiles = [ppool.tile([P, Es * P], BF16,
                                                  name=f"p_{h}", tag=f"p_{h}")
                                       for h in range(GPQ)]
                            for c in range(nch):
                                k0, k1 = c * CH, min(Es, (c + 1) * CH)
                                ncols = (k1 - k0) * P
                                adds = [kb for kb in range(k0, k1)
                                        if (s, kb) in mask_idx]
                                for h in range(GPQ):
                                    pss = sps.tile([P, 512], FP32,
                                                   name="pss", tag="pss")
                                    nc.tensor.matmul(
                                        pss[:, :ncols],
                                        qT[:, g * GPQ + h, s * P:(s + 1) * P],
                                        kTf[:, g, k0 * P:k1 * P],
                                        start=True, stop=(not adds))
                                    for na, kb in enumerate(adds):
                                        mi = mask_idx[(s, kb)]
                                        nc.tensor.matmul(
                                            pss[:, (kb - k0) * P:(kb - k0 + 1) * P],
                                            ident_bf[:], masks_t[:, mi, :],
                                            start=False, stop=(na == len(adds) - 1))
                                    nc.scalar.activation(
                                        p_tiles[h][:, k0 * P:k1 * P],
                                        pss[:, :ncols], Exp,
                                        accum_out=sums[:, h * nch + c:
                                                       h * nch + c + 1])
                            rs = astage.tile([P, GPQ], FP32, name="rs", tag="rs")
                            nc.vector.tensor_reduce(
                                rs[:],
                                sums[:].rearrange("p (h c) -> p h c", c=nch),
                                axis=mybir.AxisListType.X,
                                op=mybir.AluOpType.add)
                            rr = astage.tile([P, GPQ], FP32, name="rr", tag="rr")
                            nc.vector.reciprocal(rr[:], rs[:])
                            diags = []
                            for h in range(GPQ):
                                dg = dstage.tile([P, P], BF16,
                                                 name=f"diag{h}", tag=f"diag{h}")
                                nc.vector.tensor_scalar_mul(dg[:], ident_bf[:],
                                                            rr[:, h:h + 1])
                                diags.append(dg)
                            pctx = cps.tile([P, GPQ * P], FP32,
                                            name="pctx", tag="pctx")
                            for kb in range(Es):
                                ppt = tps.tile([P, GPQ * P], FP32,
                                               name="ppt", tag="ppt")
                                for h in range(GPQ):
                                    nc.tensor.matmul(
                                        ppt[:, h * P:(h + 1) * P],
                                        p_tiles[h][:, kb * P:(kb + 1) * P],
                                        diags[h], start=True, stop=True)
                                pts = astage.tile([P, GPQ * P], BF16,
                                                  name="pts", tag="pts")
                                nc.vector.tensor_copy(pts[:], ppt[:])
                                nc.tensor.matmul(pctx[:],
                                                 v_all[:, kb, g * HD:(g + 1) * HD],
                                                 pts[:],
                                                 start=(kb == 0),
                                                 stop=(kb == Es - 1))
                            nc.vector.tensor_copy(
                                ctxT[:, s, g, :, :],
                                pctx[:].rearrange("p (h d) -> p h d", d=P))

                # ---- phase 4: output projection ----
                with tc.tile_pool(name="ostage", bufs=3) as ostage, \
                     tc.tile_pool(name="ops", bufs=1, space="PSUM") as ops:
                    for oci, wt in ((0, wo_a), (1, wo_b)):
                        for sub in range(OC // 512):
                            po = [ops.tile([P, 512], FP32, name=f"po{s}",
                                           tag=f"po{s}") for s in range(NSLOT)]
                            for g in range(KVH):
                                for h in range(GPQ):
                                    hh = g * GPQ + h
                                    for s in range(NSLOT):
                                        nc.tensor.matmul(
                                            po[s][:], ctxT[:, s, g, h, :],
                                            wt[:, hh, sub * 512:(sub + 1) * 512],
                                            start=(hh == 0),
                                            stop=(hh == HB - 1))
                            for s in range(NSLOT):
                                ot = ostage.tile([P, 512], FP32, name="ot",
                                                 tag=f"ot{s % 2}")
                                nc.vector.tensor_copy(ot[:], po[s][:])
                                nc.sync.dma_start(
                                    out_p[s * P:(s + 1) * P,
                                          oci * OC + sub * 512:
                                          oci * OC + (sub + 1) * 512],
                                    ot[:])

    nc.compile()
    return nc


def _prep_inputs(hidden_states, attention_mask, cos, sin, Wq, Wk, Wv, Wo, P_list):
    bf16 = ml_dtypes.bfloat16
    hs = np.asarray(hidden_states, dtype=np.float32).astype(bf16)
    mask = np.asarray(attention_mask, dtype=np.float32).reshape(T, T)
    cos2 = np.asarray(cos, dtype=np.float32).reshape(T, HD)
    sin2 = np.asarray(sin, dtype=np.float32).reshape(T, HD)
    scale = np.float32(1.0 / np.sqrt(HD))

    def t3(s_):
        # rotate_half add trick: t3 = concat(sin[:, 64:], -sin[:, :64])
        return np.concatenate([s_[:, HD // 2:], -s_[:, :HD // 2]], axis=1)

    wq = np.ascontiguousarray(np.asarray(Wq, dtype=np.float32)).astype(bf16)
    wk = np.ascontiguousarray(np.asarray(Wk, dtype=np.float32)).astype(bf16)
    wv = np.ascontiguousarray(np.asarray(Wv, dtype=np.float32)).astype(bf16)
    wo = np.ascontiguousarray(np.asarray(Wo, dtype=np.float32)).astype(bf16)

    in_maps = []
    for i in range(NC):
        b, pos = i // 4, i % 4
        js = [4 * s + 3 - pos for s in range(NSLOT)]
        take = lambda a: np.ascontiguousarray(
            np.concatenate([a[j * P:(j + 1) * P] for j in js], axis=0))
        m_tiles = [mask[js[s] * P:(js[s] + 1) * P,
                        _c_of_j(c) * P:(_c_of_j(c) + 1) * P]
                   for (s, c) in P_list]
        if not m_tiles:
            m_tiles.append(np.zeros((P, P), np.float32))
        in_maps.append({
            "x": take(hs[b]),
            "wq": wq, "wk": wk, "wv": wv, "wo": wo,
            "cosq": take(cos2 * scale),
            "sinq3": take(t3(sin2 * scale)),
            "cosk": take(cos2),
            "sink3": take(t3(sin2)),
            "masks": np.stack([np.asarray(m, np.float32) for m in m_tiles]
                              ).astype(bf16),
        })
    return in_maps


_cache = {}


def kernel(hidden_states, attention_mask, cos, sin, Wq, Wk, Wv, Wo,
           _trace=False, _trace_kwargs=None):
    from concourse.bass_utils import run_bass_kernel_spmd

    E, P_list = _mask_plan(attention_mask)
    key = (tuple(E), tuple(P_list))
    if key not in _cache:
        _cache[key] = _build_program(E, P_list)
    nc = _cache[key]

    in_maps = _prep_inputs(hidden_states, attention_mask, cos, sin,
                           Wq, Wk, Wv, Wo, P_list)
    kwargs = dict(_trace_kwargs or {})
    if _trace:
        kwargs["trace"] = True
    res = run_bass_kernel_spmd(nc, in_maps, list(range(NC)), **kwargs)

    out = np.empty((B, T, HID), dtype=np.float32)
    for i in range(NC):
        b, pos = i // 4, i % 4
        o = res.results[i]["out"]
        for s in range(NSLOT):
            j = 4 * s + 3 - pos
            out[b, j * P:(j + 1) * P, :] = o[s * P:(s + 1) * P, :]
    kernel._last_result = res
    return out
